# revision 2
# baseline (speedup 1.0000x reference)
"""Token-choice MoE (8 experts, top-2, SwiGLU) on 8 Trainium2 NeuronCores.

Strategy: expert-parallel. Host computes the (tiny) router matmul + top-2
selection exactly as the reference does (jax on CPU), gathers each expert's
tokens, and ships core e the tokens routed to expert e plus that expert's
weights (pre-transposed/tiled for the TensorEngine). Each core runs the
dense SwiGLU FFN over its token batch in float32r (fp32 data, full-rate
matmul path, ~1e-4 relative error). Host scales per-token by the top-2
router weights and scatter-adds the per-expert outputs back together.

All heavy FLOPs (~825 GFLOP of matmul) run on the device; host work is the
router (0.03% of FLOPs), gathers and the final combine.
"""

import os
import numpy as np

import concourse.bass as bass
import concourse.tile as tile
from concourse import bacc, mybir
from concourse import bass_utils

# Problem geometry (hardcoded per spec)
HIDDEN = 2048
INTER = 4096
N_EXPERTS = 8
TOPK = 2
KT = HIDDEN // 128   # 16 contraction tiles for the first matmul
MT = 2 * INTER // 128  # 64 row tiles of w1 (32 gate + 32 up)
IT = INTER // 128    # 32 contraction tiles for the second matmul
HT = HIDDEN // 128   # 16 output row tiles

NT = 512             # tokens per pass (moving-operand width)

_cache = {}


def _build(C):
    """Build + compile the SPMD per-core FFN program for capacity C tokens."""
    P = C // NT
    f32r = mybir.dt.float32r
    f32 = mybir.dt.float32

    nc = bacc.Bacc("TRN2", target_bir_lowering=False, debug=False, num_devices=8)
    xt = nc.dram_tensor("xt", [128, KT, C], f32r, kind="ExternalInput").ap()
    w1t = nc.dram_tensor("w1t", [MT, 128, KT, 128], f32r, kind="ExternalInput").ap()
    w2t = nc.dram_tensor("w2t", [HT, 128, IT, 128], f32r, kind="ExternalInput").ap()
    zt = nc.dram_tensor("zt", [HT, 128, C], f32, kind="ExternalOutput").ap()

    with tile.TileContext(nc) as tc:
        with (
            tc.tile_pool(name="xt", bufs=2) as xt_pool,
            tc.tile_pool(name="w1", bufs=3) as w1_pool,
            tc.tile_pool(name="w2", bufs=2) as w2_pool,
            tc.tile_pool(name="hm", bufs=1) as hm_pool,
            tc.tile_pool(name="sg", bufs=2) as sg_pool,
            tc.tile_pool(name="out", bufs=3) as out_pool,
            tc.tile_pool(name="ps1", bufs=3, space="PSUM") as ps1,
            tc.tile_pool(name="ps2", bufs=2, space="PSUM") as ps2,
        ):
            for p in range(P):
                xt_t = xt_pool.tile([128, KT, NT], f32r)
                nc.sync.dma_start(xt_t[:], xt[:, :, p * NT:(p + 1) * NT])
                hm = hm_pool.tile([128, IT, NT], f32r)
                # First matmul + SwiGLU: pair gate row-tile m with up tile m+IT
                for m in range(IT):
                    wg = w1_pool.tile([128, KT, 128], f32r, tag="w1")
                    nc.sync.dma_start(wg[:], w1t[m])
                    wu = w1_pool.tile([128, KT, 128], f32r, tag="w1")
                    nc.sync.dma_start(wu[:], w1t[m + IT])
                    pg = ps1.tile([128, NT], f32)
                    pu = ps1.tile([128, NT], f32)
                    for k in range(KT):
                        nc.tensor.matmul(pg[:], wg[:, k, :], xt_t[:, k, :],
                                         start=(k == 0), stop=(k == KT - 1))
                    for k in range(KT):
                        nc.tensor.matmul(pu[:], wu[:, k, :], xt_t[:, k, :],
                                         start=(k == 0), stop=(k == KT - 1))
                    sg = sg_pool.tile([128, NT], f32r)
                    nc.scalar.activation(sg[:], pg[:],
                                         mybir.ActivationFunctionType.Silu)
                    nc.vector.tensor_mul(hm[:, m, :], pu[:], sg[:])
                # Second matmul: z.T[h] = sum_i w2t[h][:, i, :].T @ hm[:, i, :]
                for h in range(HT):
                    w2_t = w2_pool.tile([128, IT, 128], f32r, tag="w2")
                    nc.sync.dma_start(w2_t[:], w2t[h])
                    pz = ps2.tile([128, NT], f32)
                    for i in range(IT):
                        nc.tensor.matmul(pz[:], w2_t[:, i, :], hm[:, i, :],
                                         start=(i == 0), stop=(i == IT - 1))
                    ot = out_pool.tile([128, NT], f32)
                    nc.scalar.copy(ot[:], pz[:])
                    nc.sync.dma_start(zt[h, :, p * NT:(p + 1) * NT], ot[:])
    nc.compile()
    return nc


def kernel(hidden_states, w1, w2, router_w):
    import jax
    import jax.numpy as jnp

    orig_shape = hidden_states.shape
    x = np.ascontiguousarray(np.asarray(hidden_states).reshape(-1, HIDDEN),
                             dtype=np.float32)
    w1 = np.asarray(w1, dtype=np.float32)
    w2 = np.asarray(w2, dtype=np.float32)
    router_w = np.asarray(router_w, dtype=np.float32)
    T = x.shape[0]

    # Router on CPU, matching the reference ops exactly (jax CPU backend).
    cpu = jax.devices("cpu")[0]
    with jax.default_device(cpu):
        logits = jnp.asarray(x) @ jnp.asarray(router_w).T
        probs = jax.nn.softmax(logits.astype(jnp.float32), axis=-1)
        topk_w, sel = jax.lax.top_k(probs, TOPK)
    topk_w = np.asarray(topk_w)
    sel = np.asarray(sel)

    # Per-expert token lists and combine weights
    idxs, wts = [], []
    for e in range(N_EXPERTS):
        mask = sel == e
        tok = np.nonzero(mask.any(axis=1))[0]
        we = (topk_w * mask).sum(axis=1)[tok].astype(np.float32)
        idxs.append(tok)
        wts.append(we)

    max_count = max(len(t) for t in idxs)
    C = ((max_count + NT - 1) // NT) * NT

    key = C
    if key not in _cache:
        _cache[key] = _build(C)
    nc = _cache[key]

    # Per-core inputs: gathered+transposed tokens, tiled weights
    in_maps = []
    for e in range(N_EXPERTS):
        tok = idxs[e]
        pad = np.zeros(C, dtype=np.int64)
        pad[:len(tok)] = tok
        xg = x[pad]                                   # [C, H]
        xt_host = np.ascontiguousarray(
            xg.T.reshape(KT, 128, C).transpose(1, 0, 2))
        w1t_host = np.ascontiguousarray(
            w1[e].reshape(MT, 128, KT, 128).transpose(0, 3, 2, 1))
        w2t_host = np.ascontiguousarray(
            w2[e].reshape(HT, 128, IT, 128).transpose(0, 3, 2, 1))
        in_maps.append({"xt": xt_host, "w1t": w1t_host, "w2t": w2t_host})

    trace = bool(int(os.environ.get("MOE_KERNEL_TRACE", "0")))
    res = bass_utils.run_bass_kernel_spmd(
        nc, in_maps, core_ids=list(range(8)), trace=trace)
    kernel.last_exec_time_ns = res.exec_time_ns

    out = np.zeros_like(x)
    for e in range(N_EXPERTS):
        tok = idxs[e]
        ztile = res.results[e]["zt"]                  # [HT, 128, C]
        y = ztile.transpose(2, 0, 1).reshape(C, HIDDEN)[:len(tok)]
        out[tok] += y * wts[e][:, None]
    return out.reshape(orig_shape)


# revision 4
# speedup vs baseline: 1.0985x; 1.0985x over previous
"""Token-choice MoE (8 experts, top-2, SwiGLU) on 8 Trainium2 NeuronCores.

Strategy: expert-parallel. Host computes the (tiny) router matmul + top-2
selection exactly as the reference does (jax on CPU), gathers each expert's
tokens, and ships core e the tokens routed to expert e plus that expert's
weights (pre-transposed/tiled for the TensorEngine). Each core runs the
dense SwiGLU FFN over its token batch in float32r (fp32 data, full-rate
matmul path, ~1e-4 relative error). Host scales per-token by the top-2
router weights and scatter-adds the per-expert outputs back together.

Capacity: each core's main stage covers MAIN_C tokens (4 passes of 512).
Experts routed more than MAIN_C tokens spill into a remainder stage in the
same program: for each overflowing expert, ALL cores process that expert's
spill tokens with the expert's weights sharded 8-ways along the intermediate
dim; the host sums the 8 partial outputs.

All heavy FLOPs (~825 GFLOP of matmul) run on the device; host work is the
router (0.03% of FLOPs), gathers and the final combine.
"""

import os
import numpy as np

import concourse.bass as bass
import concourse.tile as tile
from concourse import bacc, mybir
from concourse import bass_utils

# Problem geometry (hardcoded per spec)
HIDDEN = 2048
INTER = 4096
N_EXPERTS = 8
TOPK = 2
KT = HIDDEN // 128     # 16 contraction tiles for the first matmul
MT = 2 * INTER // 128  # 64 row tiles of w1 (32 gate + 32 up)
IT = INTER // 128      # 32 contraction tiles for the second matmul
HT = HIDDEN // 128     # 16 output row tiles

NT = 512               # tokens per main pass (moving-operand width)
MAIN_P = 4             # main passes
MAIN_C = MAIN_P * NT   # main-stage capacity per expert
RIT = IT // 8          # remainder stage: i-tiles per core (inter/8)
RMT = RIT * 2          # remainder w1 row tiles per core (gate+up slices)

_cache = {}


def _build(R, RN):
    """Build + compile the SPMD per-core program.

    R remainder blocks of RN tokens each (RN == 0 when R == 0).
    """
    f32r = mybir.dt.float32r
    f32 = mybir.dt.float32

    nc = bacc.Bacc("TRN2", target_bir_lowering=False, debug=False, num_devices=8)
    xt = nc.dram_tensor("xt", [128, KT, MAIN_C], f32r, kind="ExternalInput").ap()
    w1t = nc.dram_tensor("w1t", [MT, 128, KT, 128], f32r, kind="ExternalInput").ap()
    w2t = nc.dram_tensor("w2t", [HT, 128, IT, 128], f32r, kind="ExternalInput").ap()
    zt = nc.dram_tensor("zt", [HT, 128, MAIN_C], f32, kind="ExternalOutput").ap()
    if R:
        xr = nc.dram_tensor("xr", [R, 128, KT, RN], f32r, kind="ExternalInput").ap()
        wr1 = nc.dram_tensor("wr1", [R, RMT, 128, KT, 128], f32r,
                             kind="ExternalInput").ap()
        wr2 = nc.dram_tensor("wr2", [R, HT, 128, RIT, 128], f32r,
                             kind="ExternalInput").ap()
        zr = nc.dram_tensor("zr", [R, HT, 128, RN], f32, kind="ExternalOutput").ap()

    with tile.TileContext(nc) as tc:
        with (
            tc.tile_pool(name="xt", bufs=1) as xt_pool,
            tc.tile_pool(name="w1", bufs=3) as w1_pool,
            tc.tile_pool(name="w2", bufs=2) as w2_pool,
            tc.tile_pool(name="hm", bufs=1) as hm_pool,
            tc.tile_pool(name="sg", bufs=2) as sg_pool,
            tc.tile_pool(name="out", bufs=2) as out_pool,
            tc.tile_pool(name="ps1", bufs=3, space="PSUM") as ps1,
            tc.tile_pool(name="ps2", bufs=2, space="PSUM") as ps2,
        ):
            for p in range(MAIN_P):
                xt_t = xt_pool.tile([128, KT, NT], f32r, tag="xt")
                nc.sync.dma_start(xt_t[:], xt[:, :, p * NT:(p + 1) * NT])
                hm = hm_pool.tile([128, IT, NT], f32r, tag="hm")
                # First matmul + SwiGLU: pair gate row-tile m with up tile m+IT
                for m in range(IT):
                    wg = w1_pool.tile([128, KT, 128], f32r, tag="w1")
                    nc.sync.dma_start(wg[:], w1t[m])
                    wu = w1_pool.tile([128, KT, 128], f32r, tag="w1")
                    nc.sync.dma_start(wu[:], w1t[m + IT])
                    pg = ps1.tile([128, NT], f32, tag="pg")
                    pu = ps1.tile([128, NT], f32, tag="pu")
                    for k in range(KT):
                        nc.tensor.matmul(pg[:], wg[:, k, :], xt_t[:, k, :],
                                         start=(k == 0), stop=(k == KT - 1))
                    for k in range(KT):
                        nc.tensor.matmul(pu[:], wu[:, k, :], xt_t[:, k, :],
                                         start=(k == 0), stop=(k == KT - 1))
                    sg = sg_pool.tile([128, NT], f32r, tag="sg")
                    nc.scalar.activation(sg[:], pg[:],
                                         mybir.ActivationFunctionType.Silu)
                    nc.vector.tensor_mul(hm[:, m, :], pu[:], sg[:])
                # Second matmul: z.T[h] = sum_i w2t[h][:, i, :].T @ hm[:, i, :]
                for h in range(HT):
                    w2_t = w2_pool.tile([128, IT, 128], f32r, tag="w2")
                    nc.sync.dma_start(w2_t[:], w2t[h])
                    pz = ps2.tile([128, NT], f32, tag="pz")
                    for i in range(IT):
                        nc.tensor.matmul(pz[:], w2_t[:, i, :], hm[:, i, :],
                                         start=(i == 0), stop=(i == IT - 1))
                    ot = out_pool.tile([128, NT], f32, tag="ot")
                    nc.scalar.copy(ot[:], pz[:])
                    nc.sync.dma_start(zt[h, :, p * NT:(p + 1) * NT], ot[:])

            # Remainder stage: spill tokens of overflowing experts; weights
            # inter-sharded 8-ways (this core's slice selected host-side),
            # partial z summed across cores by the host.
            for b in range(R):
                xr_t = xt_pool.tile([128, KT, RN], f32r, tag="xt")
                nc.sync.dma_start(xr_t[:], xr[b])
                hmr = hm_pool.tile([128, RIT, RN], f32r, tag="hm")
                for m in range(RIT):
                    wg = w1_pool.tile([128, KT, 128], f32r, tag="w1")
                    nc.sync.dma_start(wg[:], wr1[b, m])
                    wu = w1_pool.tile([128, KT, 128], f32r, tag="w1")
                    nc.sync.dma_start(wu[:], wr1[b, m + RIT])
                    pg = ps1.tile([128, RN], f32, tag="pg")
                    pu = ps1.tile([128, RN], f32, tag="pu")
                    for k in range(KT):
                        nc.tensor.matmul(pg[:], wg[:, k, :], xr_t[:, k, :],
                                         start=(k == 0), stop=(k == KT - 1))
                    for k in range(KT):
                        nc.tensor.matmul(pu[:], wu[:, k, :], xr_t[:, k, :],
                                         start=(k == 0), stop=(k == KT - 1))
                    sg = sg_pool.tile([128, RN], f32r, tag="sg")
                    nc.scalar.activation(sg[:], pg[:],
                                         mybir.ActivationFunctionType.Silu)
                    nc.vector.tensor_mul(hmr[:, m, :], pu[:], sg[:])
                for h in range(HT):
                    w2_t = w2_pool.tile([128, RIT, 128], f32r, tag="w2")
                    nc.sync.dma_start(w2_t[:], wr2[b, h])
                    pz = ps2.tile([128, RN], f32, tag="pz")
                    for i in range(RIT):
                        nc.tensor.matmul(pz[:], w2_t[:, i, :], hmr[:, i, :],
                                         start=(i == 0), stop=(i == RIT - 1))
                    ot = out_pool.tile([128, RN], f32, tag="ot")
                    nc.scalar.copy(ot[:], pz[:])
                    nc.sync.dma_start(zr[b, h], ot[:])
    nc.compile()
    return nc


def _tile_x(xg):
    """[C, H] fp32 -> [128, KT, C] (partition = k % 128, free = (k//128, t))."""
    C = xg.shape[0]
    return np.ascontiguousarray(xg.T.reshape(KT, 128, C).transpose(1, 0, 2))


def kernel(hidden_states, w1, w2, router_w):
    import jax
    import jax.numpy as jnp

    orig_shape = hidden_states.shape
    x = np.ascontiguousarray(np.asarray(hidden_states).reshape(-1, HIDDEN),
                             dtype=np.float32)
    w1 = np.asarray(w1, dtype=np.float32)
    w2 = np.asarray(w2, dtype=np.float32)
    router_w = np.asarray(router_w, dtype=np.float32)

    # Router on CPU, matching the reference ops exactly (jax CPU backend).
    cpu = jax.devices("cpu")[0]
    with jax.default_device(cpu):
        logits = jnp.asarray(x) @ jnp.asarray(router_w).T
        probs = jax.nn.softmax(logits.astype(jnp.float32), axis=-1)
        topk_w, sel = jax.lax.top_k(probs, TOPK)
    topk_w = np.asarray(topk_w)
    sel = np.asarray(sel)

    # Per-expert token lists and combine weights
    idxs, wts = [], []
    for e in range(N_EXPERTS):
        mask = sel == e
        tok = np.nonzero(mask.any(axis=1))[0]
        we = (topk_w * mask).sum(axis=1)[tok].astype(np.float32)
        idxs.append(tok)
        wts.append(we)

    # Remainder blocks: one per expert with more than MAIN_C tokens
    rem_experts = [e for e in range(N_EXPERTS) if len(idxs[e]) > MAIN_C]
    R = len(rem_experts)
    max_rem = max((len(idxs[e]) - MAIN_C for e in rem_experts), default=0)
    RN = max(256, ((max_rem + 127) // 128) * 128) if R else 0

    key = (R, RN)
    if key not in _cache:
        _cache[key] = _build(R, RN)
    nc = _cache[key]

    # Per-core inputs: gathered+transposed tokens, tiled weights
    in_maps = []
    for e in range(N_EXPERTS):
        tok = idxs[e][:MAIN_C]
        pad = np.zeros(MAIN_C, dtype=np.int64)
        pad[:len(tok)] = tok
        in_map = {
            "xt": _tile_x(x[pad]),
            "w1t": np.ascontiguousarray(
                w1[e].reshape(MT, 128, KT, 128).transpose(0, 3, 2, 1)),
            "w2t": np.ascontiguousarray(
                w2[e].reshape(HT, 128, IT, 128).transpose(0, 3, 2, 1)),
        }
        if R:
            c = e  # this core's inter-dim shard index
            xr_l, wr1_l, wr2_l = [], [], []
            for eb in rem_experts:
                rtok = idxs[eb][MAIN_C:]
                rpad = np.zeros(RN, dtype=np.int64)
                rpad[:len(rtok)] = rtok
                xr_l.append(_tile_x(x[rpad]))
                w1e = w1[eb].reshape(MT, 128, KT, 128).transpose(0, 3, 2, 1)
                gsel = w1e[c * RIT:(c + 1) * RIT]
                usel = w1e[IT + c * RIT:IT + (c + 1) * RIT]
                wr1_l.append(np.concatenate([gsel, usel], axis=0))
                w2e = w2[eb].reshape(HT, 128, IT, 128).transpose(0, 3, 2, 1)
                wr2_l.append(w2e[:, :, c * RIT:(c + 1) * RIT])
            in_map["xr"] = np.ascontiguousarray(np.stack(xr_l))
            in_map["wr1"] = np.ascontiguousarray(np.stack(wr1_l))
            in_map["wr2"] = np.ascontiguousarray(np.stack(wr2_l))
        in_maps.append(in_map)

    trace = bool(int(os.environ.get("MOE_KERNEL_TRACE", "0")))
    res = bass_utils.run_bass_kernel_spmd(
        nc, in_maps, core_ids=list(range(8)), trace=trace)
    kernel.last_exec_time_ns = res.exec_time_ns

    out = np.zeros_like(x)
    for e in range(N_EXPERTS):
        tok = idxs[e][:MAIN_C]
        ztile = res.results[e]["zt"]                  # [HT, 128, MAIN_C]
        y = ztile.transpose(2, 0, 1).reshape(MAIN_C, HIDDEN)[:len(tok)]
        out[tok] += y * wts[e][:len(tok), None]
    for b, eb in enumerate(rem_experts):
        rtok = idxs[eb][MAIN_C:]
        zsum = sum(res.results[c]["zr"][b] for c in range(N_EXPERTS))
        y = zsum.transpose(2, 0, 1).reshape(RN, HIDDEN)[:len(rtok)]
        out[rtok] += y * wts[eb][MAIN_C:, None]
    return out.reshape(orig_shape)
